# revision 6
# baseline (speedup 1.0000x reference)
"""Trainium2 Bass kernel for nn_AttentionLayer (shared-QK causal attention).

reference:
    Q  = X @ WQK.T
    Sc = (Q @ X^T) * sqrt(D);  Sc = where(mask, Sc, -inf)
    A  = softmax(Sc, axis=-1)
    V  = X @ WOV.T
    Y  = A @ V
    returns (X + Y, A)

Sharding: 8 cores = (batch b) x (cores-per-batch h).  Query 128-row tiles are
assigned to cores in descending order of "key prefix length" (the last key
column any row of the tile attends to, derived from the actual mask input),
interleaved so all cores share ONE uniform per-q-tile work pattern `lq` and
hence one NEFF.  Score/softmax/A^T/A@V work is skipped beyond each tile's
prefix (A is exactly zero there).  For a causal mask this cuts score-side
work ~44%; for a full mask it degrades gracefully to the dense kernel.

Precision: Q and score matmuls run in fp32 on the PE (4 cycles/row) because
the softmax is extremely peaked (score std ~1200 via the sqrt(D) multiplier)
and A is graded elementwise; fp32 PE transposes are exact.  V and A@V run
in bf16 (1 cycle/row), which only touches the X+Y output (~3e-3 relative).
"""
import sys

sys.path.insert(0, "/opt/trn_rl_repo")

import math
from contextlib import ExitStack

import numpy as np

import concourse.bass as bass
import concourse.mybir as mybir
import concourse.tile as tile
from concourse import bacc
from concourse.bass import ts
from concourse.bass_utils import run_bass_kernel_spmd
from concourse.masks import make_identity

P = 128
F32 = mybir.dt.float32
BF16 = mybir.dt.bfloat16
U8 = mybir.dt.uint8
MASK_NEG = 30000.0
N_CORES = 8


def build_nc(S, D, SH, lq):
    """One core: keys x [S, D], queries xq [SH, D] (gathered q-tiles), mask
    [SH, S] u8 (rows in kernel order, key columns natural), WQK/WOV [D, D].
    lq[j] = number of leading 128-key tiles q-tile j attends to (its A is
    exactly 0 beyond).  Outputs: out [SH, D] = xq + Y, a_out [SH, S]."""
    KD = D // P          # embedding subtiles
    NT = S // P          # key tiles
    NQ = SH // P         # query tiles
    FD = min(512, D)     # output-column block width
    FQ = min(512, SH)    # query block width (Q phase)
    NB = D // FD
    NBQ = SH // FQ
    scale = math.sqrt(D)
    assert len(lq) == NQ and all(1 <= l <= NT for l in lq)

    nc = bacc.Bacc("TRN2", target_bir_lowering=False, debug=False)
    x_d = nc.dram_tensor("x", [S, D], F32, kind="ExternalInput")
    xq_d = nc.dram_tensor("xq", [SH, D], F32, kind="ExternalInput")
    m_d = nc.dram_tensor("maskq", [SH, S], U8, kind="ExternalInput")
    wqk_d = nc.dram_tensor("wqk", [D, D], F32, kind="ExternalInput")
    wov_d = nc.dram_tensor("wov", [D, D], F32, kind="ExternalInput")
    out_d = nc.dram_tensor("out", [SH, D], F32, kind="ExternalOutput")
    a_d = nc.dram_tensor("a_out", [SH, S], F32, kind="ExternalOutput")

    with tile.TileContext(nc) as tc, ExitStack() as ctx:
        persist = ctx.enter_context(tc.tile_pool(name="persist", bufs=1))
        psum = ctx.enter_context(tc.tile_pool(name="psum", bufs=2, space="PSUM"))

        ident = persist.tile([P, P], F32, tag="ident")
        make_identity(nc, ident)

        xt = persist.tile([P, KD, S], F32, tag="xt")      # X^T [e, s]
        qt = persist.tile([P, KD, SH], F32, tag="qt")     # Q^T [e', q] (scaled)

        with tc.tile_pool(name="trans1", bufs=1) as tr1, \
             tc.tile_pool(name="loads1", bufs=3) as loads:
            # ---- X^T and XQ^T via fp32 PE transposes (exact) ----
            for st in range(NT):
                xl = loads.tile([P, D], F32, tag="ld")
                nc.sync.dma_start(xl[:], x_d[ts(st, P), :])
                for eb in range(KD):
                    pt = psum.tile([P, P], F32, tag="pt")
                    nc.tensor.transpose(pt[:], xl[:, ts(eb, P)], ident[:])
                    nc.vector.tensor_copy(xt[:, eb, ts(st, P)], pt[:])
            xqt = tr1.tile([P, KD, SH], F32, tag="xqt")
            for st in range(NQ):
                xl = loads.tile([P, D], F32, tag="ld")
                nc.sync.dma_start(xl[:], xq_d[ts(st, P), :])
                for eb in range(KD):
                    pt = psum.tile([P, P], F32, tag="pt")
                    nc.tensor.transpose(pt[:], xl[:, ts(eb, P)], ident[:])
                    nc.vector.tensor_copy(xqt[:, eb, ts(st, P)], pt[:])

            # ---- WQK^T, then QT = (WQK^T).T @ XQ^T, scaled by sqrt(D) ----
            wt = tr1.tile([P, KD, D], F32, tag="wqkt")
            for m in range(KD):
                wl = loads.tile([P, D], F32, tag="ld")
                nc.sync.dma_start(wl[:], wqk_d[ts(m, P), :])
                for eb in range(KD):
                    pt = psum.tile([P, P], F32, tag="pt")
                    nc.tensor.transpose(pt[:], wl[:, ts(eb, P)], ident[:])
                    nc.vector.tensor_copy(wt[:, eb, ts(m, P)], pt[:])
            for m in range(KD):
                for nq in range(NBQ):
                    pq = psum.tile([P, FQ], F32, tag="pacc")
                    for k in range(KD):
                        nc.tensor.matmul(
                            pq[:], wt[:, k, ts(m, P)], xqt[:, k, ts(nq, FQ)],
                            start=(k == 0), stop=(k == KD - 1),
                        )
                    nc.vector.tensor_scalar_mul(qt[:, m, ts(nq, FQ)], pq[:], scale)

        # ---- V = X @ WOV.T in bf16 (per-tile cast of X^T blocks) ----
        with tc.tile_pool(name="vpool", bufs=1) as vpool:
            vb = vpool.tile([P, NT, D], BF16, tag="vb")   # V [t, d'] bf16
            with tc.tile_pool(name="trans2", bufs=1) as tr2, \
                 tc.tile_pool(name="loads2", bufs=3) as loads2, \
                 tc.tile_pool(name="xcast", bufs=2) as xcast:
                wvb = tr2.tile([P, KD, D], BF16, tag="wvb")
                for m in range(KD):
                    wl = loads2.tile([P, D], F32, tag="ld")
                    nc.sync.dma_start(wl[:], wov_d[ts(m, P), :])
                    for eb in range(KD):
                        pt = psum.tile([P, P], F32, tag="pt")
                        nc.tensor.transpose(pt[:], wl[:, ts(eb, P)], ident[:])
                        nc.vector.tensor_copy(wvb[:, eb, ts(m, P)], pt[:])
                for mt in range(NT):
                    xc = xcast.tile([P, KD, P], BF16, tag="xc")
                    for k in range(KD):
                        nc.vector.tensor_copy(xc[:, k, :], xt[:, k, ts(mt, P)])
                    for nd in range(NB):
                        pv = psum.tile([P, FD], F32, tag="pacc")
                        for k in range(KD):
                            nc.tensor.matmul(
                                pv[:], xc[:, k, :], wvb[:, k, ts(nd, FD)],
                                start=(k == 0), stop=(k == KD - 1),
                            )
                        nc.vector.tensor_copy(vb[:, mt, ts(nd, FD)], pv[:])

            # ---- per-query-tile: scores, softmax, A^T, Y ----
            with tc.tile_pool(name="qloop", bufs=2) as ql:
                for q in range(NQ):
                    lc = lq[q] * P          # key prefix in columns
                    mk = ql.tile([P, S], U8, tag="mk")
                    nc.sync.dma_start(mk[:, :lc], m_d[ts(q, P), 0:lc])
                    sc = ql.tile([P, S], F32, tag="sc")
                    # mask bias: 0 where keep, -MASK_NEG where masked
                    nc.vector.tensor_scalar(
                        sc[:, :lc], mk[:, :lc], MASK_NEG, -MASK_NEG,
                        mybir.AluOpType.mult, mybir.AluOpType.add,
                    )
                    for t0 in range(0, lc, 512):
                        w = min(512, lc - t0)
                        ps = psum.tile([P, 512], F32, tag="psc")
                        for k in range(KD):
                            nc.tensor.matmul(
                                ps[:, :w], qt[:, k, ts(q, P)],
                                xt[:, k, t0:t0 + w],
                                start=(k == 0), stop=(k == KD - 1),
                            )
                        nc.vector.tensor_add(
                            sc[:, t0:t0 + w], ps[:, :w], sc[:, t0:t0 + w])

                    nmax = ql.tile([P, 1], F32, tag="nmax")
                    nc.vector.tensor_reduce(
                        nmax[:], sc[:, :lc], axis=mybir.AxisListType.X,
                        op=mybir.AluOpType.max, negate=True,
                    )
                    at = ql.tile([P, S], F32, tag="at")
                    sume = ql.tile([P, 1], F32, tag="sume")
                    nc.scalar.activation(
                        at[:, :lc], sc[:, :lc], mybir.ActivationFunctionType.Exp,
                        bias=nmax[:], scale=1.0, accum_out=sume[:],
                    )
                    rcp = ql.tile([P, 1], F32, tag="rcp")
                    nc.vector.reciprocal(rcp[:], sume[:])
                    nc.vector.tensor_scalar_mul(at[:, :lc], at[:, :lc], rcp[:])
                    if lc < S:
                        nc.vector.memset(at[:, lc:], 0.0)
                    nc.sync.dma_start(a_d[ts(q, P), :], at[:])

                    atb = ql.tile([P, NT, P], BF16, tag="atb")
                    for tt in range(lq[q]):
                        pt = psum.tile([P, P], F32, tag="pt")
                        nc.tensor.transpose(pt[:], at[:, ts(tt, P)], ident[:])
                        nc.vector.tensor_copy(atb[:, tt, :], pt[:])

                    xq = ql.tile([P, D], F32, tag="xq")
                    nc.sync.dma_start(xq[:], xq_d[ts(q, P), :])
                    ot = ql.tile([P, D], F32, tag="ot")
                    for nb in range(NB):
                        py = psum.tile([P, FD], F32, tag="psy")
                        for kt in range(lq[q]):
                            nc.tensor.matmul(
                                py[:], atb[:, kt, :], vb[:, kt, ts(nb, FD)],
                                start=(kt == 0), stop=(kt == lq[q] - 1),
                            )
                        nc.vector.tensor_add(
                            ot[:, ts(nb, FD)], py[:], xq[:, ts(nb, FD)])
                    nc.sync.dma_start(out_d[ts(q, P), :], ot[:])

    return nc


_cache = {}


def _get_nc(S, D, SH, lq):
    key = (S, D, SH, tuple(lq))
    if key not in _cache:
        nc = build_nc(S, D, SH, lq)
        nc.compile()
        _cache[key] = nc
    return _cache[key]


def plan(mask):
    """Derive the uniform per-q-tile key-prefix pattern and the q-tile ->
    core assignment from the actual mask values."""
    B, S, S2 = mask.shape
    cpb = N_CORES // B
    NT = S // P
    m = np.asarray(mask, dtype=bool)
    # L[g] = number of leading 128-key tiles orig q-tile g needs (max over b)
    anyk = m.reshape(B, NT, P, NT, P).any(axis=(2, 4))    # [B, NT_q, NT_k]
    anyk = anyk.any(axis=0)                               # [NT_q, NT_k]
    L = np.zeros(NT, dtype=np.int64)
    for g in range(NT):
        nz = np.nonzero(anyk[g])[0]
        L[g] = (nz[-1] + 1) if len(nz) else 1
    ranks = np.argsort(-L, kind="stable")                 # tiles by L desc
    NQ = NT // cpb
    lq = tuple(int(L[ranks[cpb * j]]) for j in range(NQ))
    # assign[h][j] = orig q-tile index for core-slot h, kernel position j
    assign = [[int(ranks[cpb * j + h]) for j in range(NQ)] for h in range(cpb)]
    return lq, assign, cpb


def make_in_maps(X, mask, WQK, WOV):
    B, S, D = X.shape
    lq, assign, cpb = plan(mask)
    SH = S // cpb
    wqk = np.ascontiguousarray(np.asarray(WQK, dtype=np.float32))
    wov = np.ascontiguousarray(np.asarray(WOV, dtype=np.float32))
    mb = np.asarray(mask)
    in_maps = []
    for c in range(N_CORES):
        b, h = divmod(c, cpb)
        gs = assign[h]
        xb = np.ascontiguousarray(np.asarray(X[b], dtype=np.float32))
        xqg = np.concatenate([xb[g * P:(g + 1) * P] for g in gs], axis=0)
        mq = np.concatenate([mb[b, g * P:(g + 1) * P] for g in gs], axis=0)
        in_maps.append({
            "x": xb,
            "xq": np.ascontiguousarray(xqg),
            "maskq": np.ascontiguousarray(mq).view(np.uint8),
            "wqk": wqk,
            "wov": wov,
        })
    return in_maps, lq, assign, SH, cpb


def assemble(results, B, S, D, assign, cpb):
    out = np.empty((B, S, D), np.float32)
    a = np.empty((B, S, S), np.float32)
    for c, r in enumerate(results):
        b, h = divmod(c, cpb)
        for j, g in enumerate(assign[h]):
            out[b, g * P:(g + 1) * P] = r["out"][j * P:(j + 1) * P]
            a[b, g * P:(g + 1) * P] = r["a_out"][j * P:(j + 1) * P]
    return out, a


def kernel(X, mask, WQK, WOV):
    B, S, D = X.shape
    in_maps, lq, assign, SH, cpb = make_in_maps(X, mask, WQK, WOV)
    nc = _get_nc(S, D, SH, lq)
    res = run_bass_kernel_spmd(nc, in_maps, core_ids=list(range(N_CORES)))
    out, a = assemble(res.results, B, S, D, assign, cpb)
    return (out, a)
